# revision 4
# baseline (speedup 1.0000x reference)
"""Trainium2 Bass kernel for nn_MultiMPNN (gnn_message_passing).

Reference computation (B=4, N=512, Z=64, E=16, H=128):
    msgs[b,i,j,:] = z[b,i]@W_i + z[b,j]@W_j + e_feat[b,i,j]@W_e + b_msg
    agg[b,i,:]    = max_j (msgs + (adj>0 ? 0 : -inf))
    out           = z@Wu_z + agg@Wu_h + b_upd

Sharding: 8 cores = (batch b, half of destination rows i).  Each core owns
IH=256 i-rows and the full j axis.

Device algorithm (v2):
 * One augmented matmul pass per row computes everything under the max:
     lhsT_aug[81,128] = [W_e ; -1e9*ones(1,H) ; W_j]          (constant)
     rhs_aug [81,w]   = [e_sel.T ; pad-mask ; z_sel.T]        (streamed bf16)
   PSUM[h, j] holds the masked messages for that row's active columns.
 * Host compacts the j axis (only adj=1 columns stream) and sorts rows by
   active count (ascending) so 4-row units share a tight width.
 * Rows are processed in periods of 4 units (16 rows, 4 PSUM banks/unit):
   3 "staged" units (ACT drains PSUM -> bf16 stage; DVE runs a 2x-mode
   tensor_tensor max tree over the 12-row supergroup) + 1 "direct" unit
   (DVE reduce_max straight off PSUM).  The split keeps ACT and DVE both
   busy; they are the only engines that can read PSUM.
 * zi + b_msg commute out of the max into the final linear; its z@Wu_z
   part is computed on the host.  Output is written [H, IH]; the host
   transposes, avoiding on-device transposes.
"""

import numpy as np
import ml_dtypes

import concourse.bacc as bacc
import concourse.mybir as mybir
import concourse.tile as tile
from concourse import bass_utils
from concourse.bass_interp import get_hw_module
from contextlib import ExitStack

B, N, Z, E, H = 4, 512, 64, 16, 128
NCORES = 8
IH = N * B // NCORES          # 256 destination rows per core
KAUG = E + 1 + Z              # 81
BANK = 512                    # f32 elems per PSUM bank

UNIT = 4                      # rows per PSUM unit (one bank per row)
PERIOD = 4                    # units per period: 3 staged + 1 direct
SGU = 3                       # staged units per supergroup (12 rows)
NUNITS = IH // UNIT           # 64
WMAX = 384                    # row width ceiling (keep-warm mm uses the tail)

F32 = mybir.dt.float32
BF16 = mybir.dt.bfloat16
NP_BF16 = ml_dtypes.bfloat16

TRACE = False                 # test.py sets True to capture an NTFF profile
TRACE_DIR = None              # optional fixed dir for trace artifacts
LAST_RESULTS = None           # BassKernelResults of the last run (for test.py)

_MODULE_CACHE = {}


def _is_direct(u):
    return (u % PERIOD) == (PERIOD - 1)


def _ensure_ntff_hook():
    """The agent image's antenv lacks axon_hooks; recreate it so
    run_bass_kernel_spmd(trace=True) can reach the axon NTFF profiler."""
    import sys
    import types

    try:
        import antenv.axon_hooks  # noqa: F401

        return
    except ImportError:
        pass
    import antenv
    from trn_agent_boot.trn_boot import _ntff_profile_via_ctypes

    state = {"h": _ntff_profile_via_ctypes("/opt/axon/libaxon_pjrt.so")}
    mod = types.ModuleType("antenv.axon_hooks")
    mod.get_axon_ntff_profile_hook = lambda: state["h"]
    mod.set_axon_ntff_profile_hook = lambda h: state.__setitem__("h", h)
    sys.modules["antenv.axon_hooks"] = mod
    antenv.axon_hooks = mod


def _build_module(widths_u):
    widths_u = list(widths_u)
    offs = [0]
    for w in widths_u:
        offs.append(offs[-1] + UNIT * w)
    tot = offs[-1]

    nc = bacc.Bacc(
        "TRN2",
        target_bir_lowering=False,
        debug=False,
        enable_asserts=False,
        num_devices=NCORES,
    )

    stream = nc.dram_tensor("stream", [KAUG, tot], BF16, kind="ExternalInput")
    lhst = nc.dram_tensor("lhst", [KAUG, H], BF16, kind="ExternalInput")
    zit = nc.dram_tensor("zit", [H, IH], F32, kind="ExternalInput")
    hostc = nc.dram_tensor("hostc", [H, IH], F32, kind="ExternalInput")
    wuh = nc.dram_tensor("wuh", [H, H], F32, kind="ExternalInput")
    out = nc.dram_tensor("out", [H, IH], F32, kind="ExternalOutput")

    with ExitStack() as ctx:
        tc = ctx.enter_context(tile.TileContext(nc))
        const = ctx.enter_context(tc.tile_pool(name="const", bufs=1))
        mega = ctx.enter_context(tc.tile_pool(name="mega", bufs=4))
        stage_pool = ctx.enter_context(tc.tile_pool(name="stage", bufs=2))
        tree_pool = ctx.enter_context(tc.tile_pool(name="tree", bufs=2))
        psum = ctx.enter_context(tc.tile_pool(name="psum", bufs=2, space="PSUM"))

        lhst_sb = const.tile([KAUG, H], BF16, tag="lhst")
        nc.scalar.dma_start(lhst_sb[:, :], lhst.ap())
        zit_sb = const.tile([H, IH], F32, tag="zit")
        nc.scalar.dma_start(zit_sb[:, :], zit.ap())
        hostc_sb = const.tile([H, IH], F32, tag="hostc")
        nc.scalar.dma_start(hostc_sb[:, :], hostc.ap())
        wuh_sb = const.tile([H, H], F32, tag="wuh")
        nc.scalar.dma_start(wuh_sb[:, :], wuh.ap())

        magg = const.tile([H, IH], F32, tag="magg")
        warm_a = const.tile([H, BANK], BF16, tag="warm_a")
        nc.vector.memset(warm_a[:, :], 0.0)

        # PE warm-up burn: open the HAM clock gate during DMA-bound startup.
        pw = psum.tile([H, UNIT * BANK], F32, tag="ps")
        for _ in range(8):
            nc.tensor.matmul(
                pw[:, :BANK], warm_a[:, :H], warm_a[:, :], start=True, stop=True
            )

        stream_ap = stream.ap()

        # DMA blocks: whole periods; small first blocks for ramp-up.
        pblocks = [[0], [1], [2, 3], [4, 5]]
        p = 6
        while p < NUNITS // PERIOD:
            pblocks.append(list(range(p, min(p + 2, NUNITS // PERIOD))))
            p += 2

        for pblock in pblocks:
            u0, u1 = pblock[0] * PERIOD, (pblock[-1] + 1) * PERIOD
            b0, b1 = offs[u0], offs[u1]
            mb = mega.tile([KAUG, b1 - b0], BF16, tag="mega")
            nc.sync.dma_start(mb[:, :], stream_ap[:, b0:b1])

            for period in pblock:
                # staged supergroup for this period
                sg_w = widths_u[period * PERIOD]
                sg_stage = stage_pool.tile([H, SGU * UNIT * sg_w], BF16, tag="st")
                sg_row0 = period * PERIOD * UNIT

                for k in range(PERIOD):
                    u = period * PERIOD + k
                    w = widths_u[u]
                    row0 = u * UNIT
                    uoff = offs[u] - b0
                    ps = psum.tile([H, UNIT * BANK], F32, tag="ps")
                    ps3 = ps[:, :].rearrange("p (g j) -> p g j", g=UNIT)
                    for r in range(UNIT):
                        nc.tensor.matmul(
                            ps3[:, r, :w],
                            lhst_sb[:, :],
                            mb[:, uoff + r * w: uoff + (r + 1) * w],
                            start=True,
                            stop=True,
                        )
                    # keep-warm: tail of the last bank (w <= WMAX < 384+128)
                    nc.tensor.matmul(
                        ps3[:, 3, BANK - H:],
                        warm_a[:, :H],
                        warm_a[:, :H],
                        start=True,
                        stop=True,
                    )
                    if _is_direct(u):
                        nc.vector.reduce_max(
                            magg[:, row0: row0 + UNIT],
                            ps3[:, :, :w],
                            axis=mybir.AxisListType.X,
                        )
                    else:
                        assert w == sg_w
                        st = sg_stage[
                            :, k * UNIT * sg_w: (k + 1) * UNIT * sg_w
                        ].rearrange("p (g j) -> p g j", g=UNIT)
                        nc.scalar.copy(st[:, :, :], ps3[:, :, :sg_w])

                # tree over the 12-row supergroup
                g = SGU * UNIT
                w = sg_w
                st = sg_stage[:, :].rearrange("p (g j) -> p g j", g=g)
                l2 = tree_pool.tile([H, g * (WMAX // 2)], BF16, tag="l2")
                l2r = l2[:, : g * (w // 2)].rearrange("p (g j) -> p g j", g=g)
                nc.vector.tensor_tensor(
                    l2r[:, :, :], st[:, :, : w // 2], st[:, :, w // 2:],
                    mybir.AluOpType.max,
                )
                l3 = tree_pool.tile([H, g * (WMAX // 4)], BF16, tag="l3")
                l3r = l3[:, : g * (w // 4)].rearrange("p (g j) -> p g j", g=g)
                nc.vector.tensor_tensor(
                    l3r[:, :, :], l2r[:, :, : w // 4], l2r[:, :, w // 4:],
                    mybir.AluOpType.max,
                )
                l4 = tree_pool.tile([H, g * (WMAX // 8)], BF16, tag="l4")
                l4r = l4[:, : g * (w // 8)].rearrange("p (g j) -> p g j", g=g)
                nc.vector.tensor_tensor(
                    l4r[:, :, :], l3r[:, :, : w // 8], l3r[:, :, w // 8:],
                    mybir.AluOpType.max,
                )
                nc.vector.reduce_max(
                    magg[:, sg_row0: sg_row0 + g],
                    l4r[:, :, :],
                    axis=mybir.AxisListType.X,
                )

        # final linear: out = Wu_h.T @ (magg + zit) + hostc   (layout [H, IH])
        aggt = const.tile([H, IH], F32, tag="aggt")
        nc.vector.tensor_add(aggt[:, :], magg[:, :], zit_sb[:, :])
        psf = psum.tile([H, UNIT * BANK], F32, tag="ps")
        nc.tensor.matmul(psf[:, :IH], wuh_sb[:, :], aggt[:, :], start=True, stop=True)
        outt = const.tile([H, IH], F32, tag="outt")
        nc.vector.tensor_add(outt[:, :], psf[:, :IH], hostc_sb[:, :])
        nc.sync.dma_start(out.ap()[:, :], outt[:, :])

    nc.compile()
    nc.m = get_hw_module(nc.m)
    return nc


def _unit_widths(csort):
    """Shared width per 4-row unit; staged units in a period share the
    period's staged max so the supergroup tree APs are uniform."""
    umax = np.zeros(NUNITS, dtype=int)
    for u in range(NUNITS):
        umax[u] = csort[:, u * UNIT: (u + 1) * UNIT].max()
    widths_u = np.clip((umax + 7) // 8 * 8, 16, N).astype(int)
    for p in range(NUNITS // PERIOD):
        mem = [p * PERIOD, p * PERIOD + 1, p * PERIOD + 2]
        widths_u[mem] = widths_u[mem].max()
    assert widths_u.max() <= WMAX, widths_u.max()
    return widths_u


def _prepare(z, e_feat, adj, W_msg, b_msg, W_upd, b_upd):
    """Host-side sharding + compaction (rows sorted by count, ascending)."""
    W_i, W_j, W_e = W_msg[:Z], W_msg[Z: 2 * Z], W_msg[2 * Z:]
    Wu_z, Wu_h = W_upd[:Z], W_upd[Z:]

    counts = (adj > 0).sum(axis=-1)                   # [B, N]
    orders, csort = [], []
    for c in range(NCORES):
        b, half = divmod(c, NCORES // B)
        cnt = counts[b, half * IH: (half + 1) * IH]
        order = np.argsort(cnt, kind="stable")
        orders.append(order)
        csort.append(cnt[order])
    csort = np.stack(csort)                           # [NCORES, IH]

    widths_u = _unit_widths(csort)
    offs = np.concatenate([[0], np.cumsum([UNIT * w for w in widths_u])])
    tot = int(offs[-1])
    maxw = int(widths_u.max())

    lhst_np = np.concatenate(
        [W_e, np.full((1, H), -1e9, np.float32), W_j], axis=0
    ).astype(NP_BF16)
    wuh_np = np.ascontiguousarray(Wu_h, np.float32)

    in_maps = []
    for c in range(NCORES):
        b, half = divmod(c, NCORES // B)
        sl = slice(half * IH, (half + 1) * IH)
        order = orders[c]
        adj_blk = (adj[b, sl] > 0)[order]             # [IH, N] sorted rows
        jorder = np.argsort(~adj_blk, axis=-1, kind="stable")[:, :maxw]
        e_sel = np.take_along_axis(
            e_feat[b, sl][order], jorder[:, :, None], axis=1
        )                                             # [IH, maxw, E]
        z_sel = z[b][jorder]                          # [IH, maxw, Z]
        msk = ~np.take_along_axis(adj_blk, jorder, axis=1)

        stream = np.empty((KAUG, tot), dtype=NP_BF16)
        for u in range(NUNITS):
            w = widths_u[u]
            for k in range(UNIT):
                r = u * UNIT + k
                o = offs[u] + k * w
                stream[:E, o: o + w] = e_sel[r, :w].T
                stream[E, o: o + w] = msk[r, :w]
                stream[E + 1:, o: o + w] = z_sel[r, :w].T

        zperm = z[b, sl][order]
        in_maps.append(
            {
                "stream": stream,
                "lhst": lhst_np,
                "zit": np.ascontiguousarray(
                    (zperm @ W_i).T + b_msg[:, None], dtype=np.float32
                ),
                "hostc": np.ascontiguousarray(
                    (zperm @ Wu_z + b_upd).T, dtype=np.float32
                ),
                "wuh": wuh_np,
            }
        )
    return in_maps, widths_u, orders


def kernel(z, e_feat, adj, W_msg, b_msg, W_upd, b_upd):
    global LAST_RESULTS

    z = np.asarray(z, np.float32)
    e_feat = np.asarray(e_feat, np.float32)
    adj = np.asarray(adj)
    W_msg = np.asarray(W_msg, np.float32)
    b_msg = np.asarray(b_msg, np.float32)
    W_upd = np.asarray(W_upd, np.float32)
    b_upd = np.asarray(b_upd, np.float32)

    in_maps, widths_u, orders = _prepare(
        z, e_feat, adj, W_msg, b_msg, W_upd, b_upd
    )

    key = tuple(widths_u)
    if key not in _MODULE_CACHE:
        _MODULE_CACHE[key] = _build_module(widths_u)
    nc = _MODULE_CACHE[key]

    if TRACE:
        _ensure_ntff_hook()
    res = bass_utils.run_bass_kernel_spmd(
        nc, in_maps, core_ids=list(range(NCORES)), trace=TRACE, tmpdir=TRACE_DIR
    )
    LAST_RESULTS = res

    full = np.empty((B, N, H), np.float32)
    for c in range(NCORES):
        b, half = divmod(c, NCORES // B)
        full[b, half * IH + orders[c]] = res.results[c]["out"].T
    return full


if __name__ == "__main__":
    rng = np.random.default_rng(0)
    ins = {
        "z": rng.standard_normal((B, N, Z)).astype(np.float32),
        "e_feat": rng.standard_normal((B, N, N, E)).astype(np.float32),
        "adj": (rng.random((B, N, N)) < 0.5).astype(np.int32),
        "W_msg": (rng.standard_normal((2 * Z + E, H)) * 0.1).astype(np.float32),
        "b_msg": np.zeros(H, np.float32),
        "W_upd": (rng.standard_normal((Z + H, H)) * 0.1).astype(np.float32),
        "b_upd": np.zeros(H, np.float32),
    }
    out = kernel(**ins)
    print("out", out.shape, out.dtype, float(np.abs(out).max()))
